# revision 1
# baseline (speedup 1.0000x reference)
"""Multi-head attention block (B=2, N=2048, C=1024, H=16, D=64) on 8 TRN2
NeuronCores.

Sharding: tensor-parallel over heads - 2 heads per core, both batch elements.
Each core computes qkv for its 2 heads, full attention for its 4 (batch, head)
pairs, and a partial output projection over its 128 columns of the attention
output. The host sums the 8 fp16 partial projections and adds the bias.

v3 design (vs the 300us baseline):
  - S matmuls row-tiled across the head pair: kT/qT keep head0 on partitions
    0-63 and head1 on 64-127, so the two K=64 S matmuls land on disjoint PE
    row-groups (tile_position (0,0)/(64,0) auto-derived from base partitions)
    and run concurrently - S cost halves.
  - j-major attention steps: each step computes ST for 512 queries x 128 keys
    for BOTH heads into one [128,1024] PSUM pair tile, one [128,1024] exp on
    ACT (the overall bottleneck: 128 exps ~= 147us), then two M=65 PV matmuls
    (V' carries a ones column so the softmax denominator accumulates free).
    PV lags one step behind exp so the in-order PE queue never waits.
  - V' is computed directly token-major (x chunk as stationary) - no PE
    transposes at all.
  - PSUM budget exactly 8 banks: ST pair [128,1024]x2 + OT [65,512]x2 +
    filler [128,512]x2 (qkv/V'/proj/warmup share the filler pool).
  - HAM clock gate: warm-up matmuls at t=0, and a credit-based fill queue
    paces qkv/proj work into the per-step PE slack so the PE never idles
    long enough to re-throttle to K=4/8.
  - Normalization: denominator reciprocal via reciprocal_approx_fast (DVE),
    partition_broadcast + multiply on GpSimd (keeps DVE for PSUM evictions;
    GpSimd has no PSUM port).
  - y partials in fp16 (halves output DMA); host sums in f32.
"""
import sys

sys.path.insert(0, "/opt/trn_rl_repo")

import numpy as np

B = 2
N = 2048
C = 1024
H = 16
D = 64
R = B * N            # 4096 flattened rows
NCORES = 8
HPC = H // NCORES    # heads per core = 2
SCALE = 1.0 / np.sqrt(D)  # 0.125

_NC_CACHE = None


def build_nc():
    import concourse.bass as bass
    import concourse.tile as tile
    from concourse import bacc, mybir

    F32 = mybir.dt.float32
    FP16 = mybir.dt.float16
    Exp = mybir.ActivationFunctionType.Exp

    nc = bacc.Bacc("TRN2", target_bir_lowering=False, debug=False,
                   num_devices=NCORES)

    xT_d = nc.declare_dram_parameter("xT", [C, R], FP16, isOutput=False)
    wqkvT_d = nc.declare_dram_parameter("wqkvT", [C, 3 * 2 * D], FP16,
                                        isOutput=False)
    wprojT_d = nc.declare_dram_parameter("wprojT", [2 * D, C], FP16,
                                         isOutput=False)
    y_d = nc.declare_dram_parameter("y", [R, C], FP16, isOutput=True)

    CC = C // 128    # 8 contraction chunks
    NMC = N // 128   # 16 key chunks per batch

    with tile.TileContext(nc) as tc:
        with (
            tc.tile_pool(name="const", bufs=1) as const,
            tc.tile_pool(name="qkvT", bufs=1) as qkvp,
            tc.tile_pool(name="vprime", bufs=1) as vpp,
            tc.tile_pool(name="otbuf", bufs=1) as otp,
            tc.tile_pool(name="xt", bufs=4) as xtp,
            tc.tile_pool(name="et", bufs=3) as etp,
            tc.tile_pool(name="small", bufs=4) as small,
            tc.tile_pool(name="ysb", bufs=4) as ysbp,
            tc.tile_pool(name="stp", bufs=2, space="PSUM") as stp,
            tc.tile_pool(name="fillp", bufs=2, space="PSUM") as fillp,
            tc.tile_pool(name="otps", bufs=1, space="PSUM") as otps,
        ):
            # ---- constants ----
            wqkv_sb = const.tile([128, CC, 3 * 2 * D], FP16)
            wproj_sb = const.tile([128, C], FP16)
            warm = const.tile([128, 512], FP16)

            # ---- persistent activations ----
            qT = qkvp.tile([128, R], FP16)   # rows: h0 d-major | h1 d-major
            kT = qkvp.tile([128, R], FP16)
            vprime = [[vpp.tile([128, NMC, D + 1], FP16, tag=f"vp{b}{hl}",
                                name=f"vp{b}{hl}")
                       for hl in range(HPC)] for b in range(B)]
            # normalized attn out, c-major; one tile per (b, qh, j) 512-token
            # block so the tile-granular dep tracker never makes a proj wait
            # on an unrelated block's norm writes
            ot = {(b, qh, j): otp.tile([128, 512], FP16,
                                       tag=f"ot{b}{qh}{j}",
                                       name=f"ot{b}{qh}{j}")
                  for b in range(B) for qh in range(2) for j in range(2)}

            for b in range(B):
                for hl in range(HPC):
                    nc.gpsimd.memset(vprime[b][hl][:, :, D:D + 1], 1.0)

            # ---- building blocks ----
            xts = {}

            def xt_load(rb, eng=None):
                xt = xtp.tile([128, CC, 512], FP16, tag="xt", name="xt")
                col0 = rb * 512
                (eng or nc.sync).dma_start(
                    xt[:],
                    xT_d[:, col0:col0 + 512].rearrange(
                        "(a p) r -> p a r", p=128))
                xts[rb] = xt

            def qk_half(rb, ob, lo, state):
                # half of a q/k chain (4 contraction chunks, ~1us of PE) so
                # fill units never delay the S/exp cadence by more than that
                col0 = rb * 512
                dst = (qT, kT)[ob]
                if lo == 0:
                    state["ps"] = fillp.tile([128, 512], F32, tag="fill",
                                             name="qkps")
                ps = state["ps"]
                for cc in range(lo, lo + CC // 2):
                    nc.tensor.matmul(
                        ps[:],
                        wqkv_sb[:, cc, ob * 128:(ob + 1) * 128],
                        xts[rb][:, cc, :],
                        start=(cc == 0), stop=(cc == CC - 1),
                    )
                if lo:
                    nc.vector.tensor_copy(dst[:, col0:col0 + 512], ps[:])

            def qk_group(rb, ob):
                st = {}
                qk_half(rb, ob, 0, st)
                qk_half(rb, ob, CC // 2, st)

            def vprime_chunk(b, mc):
                # V' for one 128-token chunk, token-major, both heads at once
                rb = (b * N + mc * 128) // 512
                tok0 = (b * N + mc * 128) % 512
                ps = fillp.tile([128, 512], F32, tag="fill", name="vpps")
                for cc in range(CC):
                    nc.tensor.matmul(
                        ps[:, 0:128],
                        xts[rb][:, cc, tok0:tok0 + 128],
                        wqkv_sb[:, cc, 2 * 128:3 * 128],
                        start=(cc == 0), stop=(cc == CC - 1),
                    )
                for hl in range(HPC):
                    nc.vector.tensor_copy(
                        vprime[b][hl][:, mc, 0:D],
                        ps[:, hl * D:(hl + 1) * D])

            otus = {}

            def evict_ot(b, qh, j, hl, ot_ps):
                otu = small.tile([D + 1, 512], F32, tag=f"otu{hl}",
                                 name="otu")
                nc.vector.tensor_copy(otu[:], ot_ps[:])
                otus[(b, qh, j, hl)] = otu

            def norm_unit(b, qh, j, hl):
                # baseline-proven datapath: chunked reciprocal + final mul on
                # DVE, partition broadcast on GpSimd
                p0 = hl * D
                q0 = b * N + qh * 1024 + j * 512

                def _recip(ch):
                    if ch == 0:
                        rinv = small.tile([1, 512], F32, tag="rinv",
                                          name="rinv")
                        otus[(b, qh, j, hl)] = (otus[(b, qh, j, hl)], rinv)
                    otu, rinv = otus[(b, qh, j, hl)]
                    nc.vector.reciprocal(
                        rinv[:, ch * 256:(ch + 1) * 256],
                        otu[D:D + 1, ch * 256:(ch + 1) * 256])

                def _mul():
                    otu, rinv = otus.pop((b, qh, j, hl))
                    rbig = small.tile([D, 512], F32, tag="rbig", name="rbig")
                    nc.gpsimd.partition_broadcast(rbig[:], rinv[:])
                    nc.vector.tensor_mul(
                        ot[(b, qh, j)][p0:p0 + D, :], otu[0:D, :], rbig[:])

                return [lambda ch=ch: _recip(ch) for ch in range(2)] + [_mul]

            def normalize_act(b, qh, j, hl):
                # tail-only: reciprocal via exp(-ln(d)) on ACT, which is idle
                # after the last attention exp, instead of the 2x1.7us DVE
                # reciprocal chain
                p0 = hl * D
                otu = otus.pop((b, qh, j, hl))
                lnd = small.tile([1, 512], F32, tag="lnd", name="lnd")
                nc.scalar.activation(lnd[:], otu[D:D + 1, :],
                                     mybir.ActivationFunctionType.Ln)
                rinv = small.tile([1, 512], F32, tag="rinva", name="rinva")
                nc.scalar.activation(rinv[:], lnd[:], Exp, scale=-1.0)
                rbig = small.tile([D, 512], F32, tag="rbiga", name="rbiga")
                nc.gpsimd.partition_broadcast(rbig[:], rinv[:])
                nc.vector.tensor_mul(
                    ot[(b, qh, j)][p0:p0 + D, :], otu[0:D, :], rbig[:])

            ysbs = {}

            def proj_unit(rb, j, eng="v"):
                # partial y for one 128-token block, 512 output cols
                ps = fillp.tile([128, 512], F32, tag="fill", name="yp")
                src = ot[(rb // 16, (rb // 8) % 2, (rb // 4) % 2)]
                col0 = (rb % 4) * 128
                nc.tensor.matmul(
                    ps[:],
                    src[:, col0:col0 + 128],
                    wproj_sb[:, j * 512:(j + 1) * 512],
                    start=True, stop=True,
                )
                if rb not in ysbs:
                    ysbs[rb] = ysbp.tile([128, C], FP16, tag="ysb",
                                         name="ysb")
                ysb = ysbs[rb]
                if eng == "v":
                    nc.vector.tensor_copy(ysb[:, j * 512:(j + 1) * 512],
                                          ps[:])
                else:
                    nc.scalar.copy(ysb[:, j * 512:(j + 1) * 512], ps[:])
                if j == 1:
                    nc.sync.dma_start(y_d[rb * 128:(rb + 1) * 128, :],
                                      ysb[:])
                    del ysbs[rb]

            # ---- fill queue: paces PE-filler work into per-step slack.
            # Emission order defines RAW deps, so consumers force-drain the
            # queue up to their producer's key before emitting (need()).
            class FillQueue:
                def __init__(self):
                    self.units = []   # (cost_ns, fn, key)
                    self.i = 0
                    self.credit = 2600.0
                    self.done = set()

                def add(self, cost, fn, key=None):
                    self.units.append((cost, fn, key))

                def _run(self):
                    cost, fn, key = self.units[self.i]
                    fn()
                    if key is not None:
                        self.done.add(key)
                    self.i += 1
                    return cost

                def step(self, slack):
                    self.credit = min(self.credit + slack, 2600.0)
                    while self.i < len(self.units):
                        if self.units[self.i][0] > self.credit:
                            break
                        self.credit -= self._run()

                def need(self, key):
                    if key in self.done:
                        return
                    assert any(u[2] == key for u in self.units[self.i:]), key
                    while key not in self.done:
                        self._run()

                def drain(self):
                    while self.i < len(self.units):
                        self._run()

            fq = FillQueue()

            # ---- attention pipeline ----
            pend = [None]   # PV one step behind exp, carried across halves

            def flush_pend():
                if pend[0] is None:
                    return
                b_, qh_, j_, mc_, et_, ops_ = pend[0]
                pend[0] = None
                fq.need(("v", b_, mc_))
                for hl in range(HPC):
                    nc.tensor.matmul(
                        ops_[hl][:],
                        vprime[b_][hl][:, mc_, :],
                        et_[:, hl * 512:(hl + 1) * 512],
                        start=(mc_ == 0), stop=(mc_ == NMC - 1),
                    )
                if mc_ == NMC - 1:
                    for hl in range(HPC):
                        evict_ot(b_, qh_, j_, hl, ops_[hl])
                    # norm work for this j becomes available now; it runs on
                    # DVE/GpSimd during the next ~16 steps, well before any
                    # proj unit for these tokens reaches the PE queue. The
                    # final block's norm instead runs on ACT in the tail.
                    if (b_, qh_, j_) != (0, 1, 1):
                        for hl in range(HPC):
                            for u in norm_unit(b_, qh_, j_, hl):
                                fq.add(0, u)

            def add_proj(rbs):
                for rb in rbs:
                    for j in range(2):
                        fq.add(300, lambda rb=rb, j=j: proj_unit(rb, j))

            def attention_half(b, qh, slack=510.0, proj_start=(),
                               proj_mid=()):
                # proj units queue BEFORE flush_pend's norm units: the dep
                # tracker is tile-granular on `ot`, so a proj emitted after
                # fresher norm muls would wait on them (and stall the PE
                # queue behind it)
                add_proj(proj_start)
                flush_pend()
                q0 = b * N + qh * 1024
                for j in range(2):
                    ot_ps = [otps.tile([D + 1, 512], F32, tag=f"ot{hl}",
                                       name=f"otps{hl}")
                             for hl in range(HPC)]
                    qcol = q0 + j * 512
                    fq.need(("q", qcol // 512))
                    for mc in range(NMC):
                        if j == 1 and mc == 8:
                            # previous half's j1 tokens: their norm chain was
                            # emitted at this half's start and needs ~8 steps
                            # of DVE time before proj can enter the PE queue
                            add_proj(proj_mid)
                        fq.step(slack)
                        kcol = b * N + mc * 128
                        fq.need(("k", kcol // 512))
                        st = stp.tile([128, 1024], F32, tag="stp",
                                      name="st")
                        for hl in range(HPC):
                            nc.tensor.matmul(
                                st[:, hl * 512:(hl + 1) * 512],
                                kT[hl * D:(hl + 1) * D, kcol:kcol + 128],
                                qT[hl * D:(hl + 1) * D, qcol:qcol + 512],
                                start=True, stop=True,
                            )
                        et = etp.tile([128, 1024], FP16, tag="et", name="et")
                        nc.scalar.activation(et[:], st[:], Exp, scale=SCALE)
                        flush_pend()
                        pend[0] = (b, qh, j, mc, et, ot_ps)

            # ---- emission ----
            with nc.named_scope("startup"):
                # DMA order matters: transfers serialize on the SP queue,
                # so the first half's dependencies (wqkv as ONE transfer,
                # then xt4..7) go first and wproj (not needed until the
                # first proj, ~60us in) last.
                # xt5-7 and wproj issue from the ACT queue (idle until the
                # first exp) so their descriptor generation runs in parallel
                # with the SP queue's wqkv+xt4
                wq_r = wqkvT_d.rearrange("(a p) o -> p a o", p=128)
                nc.sync.dma_start(wqkv_sb[:], wq_r[:])
                xt_load(4)
                for rb in (5, 6, 7):
                    xt_load(rb, eng=nc.scalar)
                nc.scalar.dma_start(wproj_sb[:], wprojT_d[:])
                # HAM warm-up: back-to-back matmuls on a memset tile lift
                # the PE clock gate to K=8/8 while the DMAs land, so the
                # first qkv chains run at 2.4 GHz
                nc.gpsimd.memset(warm[:], 0.125)
                for w in range(12):
                    ps = fillp.tile([128, 512], F32, tag="fill", name="warm")
                    nc.tensor.matmul(ps[:], warm[:, 0:128], warm[:],
                                     start=True, stop=True)
                # minimum work for the first attention half (b=1, qh=0)
                qk_group(4, 1)
                qk_group(4, 0)
            fq.done.update([("k", 4), ("q", 4)])

            # global ordered fill list; hw deps gate execution, the queue
            # only paces emission into PE slack. Order follows need time in
            # the attn10 pipeline; need() force-drains stragglers.
            def add_qk(rb, ob, key):
                st = {}
                fq.add(1000, lambda: qk_half(rb, ob, 0, st))
                fq.add(1000, lambda: qk_half(rb, ob, CC // 2, st), key)

            add_qk(5, 1, ("k", 5))
            for mc in range(8):
                fq.add(600, lambda mc=mc: vprime_chunk(1, mc), ("v", 1, mc))
            add_qk(6, 1, ("k", 6))
            add_qk(7, 1, ("k", 7))
            for mc in range(8, 16):
                fq.add(600, lambda mc=mc: vprime_chunk(1, mc), ("v", 1, mc))
            add_qk(5, 0, ("q", 5))
            add_qk(6, 0, ("q", 6))
            add_qk(7, 0, ("q", 7))
            for rb in range(4):
                fq.add(0, lambda rb=rb: xt_load(rb))
                add_qk(rb, 1, ("k", rb))
                for mc in range(4 * rb, 4 * rb + 4):
                    fq.add(600, lambda mc=mc: vprime_chunk(0, mc),
                           ("v", 0, mc))
                add_qk(rb, 0, ("q", rb))

            with nc.named_scope("attn10"):
                attention_half(1, 0, slack=800.0)
            with nc.named_scope("attn11"):
                attention_half(1, 1, proj_start=range(16, 20),
                               proj_mid=range(20, 24))
            with nc.named_scope("attn00"):
                attention_half(0, 0, proj_start=range(24, 28),
                               proj_mid=range(28, 32))
            with nc.named_scope("attn01"):
                attention_half(0, 1, proj_start=range(0, 4),
                               proj_mid=range(4, 8))

            with nc.named_scope("tail"):
                fq.drain()
                # rb 8-11 only need norm(0,1,j0) (ran mid-attn01); keep the
                # PE busy on them while the last exp + PV finish
                for rb in range(8, 12):
                    proj_unit(rb, 0)
                    proj_unit(rb, 1)
                flush_pend()
                fq.drain()
                # last block's norm on ACT (idle now); two parallel chains
                normalize_act(0, 1, 1, 0)
                normalize_act(0, 1, 1, 1)
                for i, rb in enumerate(range(12, 16)):
                    proj_unit(rb, 0, eng="v" if i % 2 else "s")
                    proj_unit(rb, 1, eng="s" if i % 2 else "v")

    nc.compile()
    return nc


def get_nc():
    global _NC_CACHE
    if _NC_CACHE is None:
        _NC_CACHE = build_nc()
    return _NC_CACHE


def make_in_maps(x, w_qkv, w_proj):
    x = np.asarray(x, dtype=np.float32)
    w_qkv = np.asarray(w_qkv, dtype=np.float32)
    w_proj = np.asarray(w_proj, dtype=np.float32)
    xT = np.ascontiguousarray(x.reshape(R, C).T.astype(np.float16))
    in_maps = []
    for i in range(NCORES):
        h0, h1 = HPC * i, HPC * i + 1
        rows = []
        for part in range(3):  # q, k, v
            for h in (h0, h1):
                lo = part * C + h * D
                rows.append(w_qkv[lo:lo + D])
        w_slice = np.concatenate(rows, axis=0)           # [384, 1024]
        wqkvT = np.ascontiguousarray(w_slice.T.astype(np.float16))
        cols = np.r_[h0 * D:(h0 + 1) * D, h1 * D:(h1 + 1) * D]
        wprojT = np.ascontiguousarray(w_proj[:, cols].T.astype(np.float16))
        in_maps.append({"xT": xT, "wqkvT": wqkvT, "wprojT": wprojT})
    return in_maps


def kernel(x, w_qkv, w_proj, b_proj):
    from concourse.bass_utils import run_bass_kernel_spmd

    nc = get_nc()
    in_maps = make_in_maps(x, w_qkv, w_proj)
    res = run_bass_kernel_spmd(nc, in_maps, core_ids=list(range(NCORES)))
    y = np.zeros((R, C), dtype=np.float32)
    for r in res.results:
        y += np.asarray(r["y"], dtype=np.float32)
    y += np.asarray(b_proj, dtype=np.float32)[None, :]
    return y.reshape(B, N, C)

